# revision 3
# baseline (speedup 1.0000x reference)
"""LSTM-pool kernel for Trainium2, 8-core data-parallel SPMD.

Math (per batch row b):
  x_t = [seq[b,t], seq_e[b,t], seq_t[b,t]]              (A = 384)
  z_t = x_t @ Wi + h_{t-1} @ Wh + bh                    (4F = 512, gates i,f,g,o)
  c_t = sig(f)*c_{t-1} + sig(i)*tanh(g);  h_t = sig(o)*tanh(c_t)
  out = relu([h_T, src] @ W1 + b1) @ W2 + b2

Key design points (vs v1):
  * Host pre-transposes and pre-casts x to bf16 into the exact SBUF layout
    the matmuls need, so the device does one clean HWDGE DMA per chunk
    (no SWDGE cast pass, no xbar-transpose pass -> ~2.5x less DMA work).
  * Gate quads reordered to [g, f, i, o] (host-side weight column perm).
  * tanh(g) computed as 2*sigmoid(2g)-1 with the 2x folded into the g-quad
    weight columns on the host (exact in bf16), so ONE sigmoid instruction
    covers all four gate quads -> ACT drops from 6 to 4 ops per step.
  * All gates + cell state in bf16 -> DVE tensor_tensor runs in 2x mode.
  * The two batch halves are software-pipelined half a step apart via
    explicit emission order (Tile priorities), instead of running lockstep.
"""

import sys

sys.path.insert(0, "/opt/trn_rl_repo")

import numpy as np
import ml_dtypes

import concourse.bass as bass
import concourse.mybir as mybir
import concourse.tile as tile
from concourse import bacc
from concourse.bass_utils import run_bass_kernel_spmd

dt = mybir.dt
AF = mybir.ActivationFunctionType

NCORES = 8
BFULL = 4096
B = BFULL // NCORES  # 512 batch rows per core
T = 128
F = 128
G = 512  # 4F
TC = 8  # time steps per DMA chunk
NCHUNK = T // TC
NH = B // 2  # half-batch = 256
PREFETCH = 3  # xt chunk buffers in flight

# z quadrant order on device: [g, f, i, o] (original Wi/Wh order: i,f,g,o).
# Host permutes weight columns so device sees this order.
QPERM = [2, 1, 0, 3]  # device quad q comes from original gate QPERM[q]
QG, QF, QI, QO = 0, 1, 2, 3


def build_nc(zero_bias: bool):
    nc = bacc.Bacc("TRN2", target_bir_lowering=False, debug=False, num_devices=NCORES)

    xt_d = nc.dram_tensor("xt", [NCHUNK, 128, 3, TC, B], dt.bfloat16, kind="ExternalInput")
    srcT_d = nc.dram_tensor("srcT", [128, B], dt.bfloat16, kind="ExternalInput")
    wi_d = nc.dram_tensor("wi", [128, 3, G], dt.bfloat16, kind="ExternalInput")
    wh_d = nc.dram_tensor("wh", [128, G], dt.bfloat16, kind="ExternalInput")
    bhq_d = nc.dram_tensor("bhq", [128, 4], dt.float32, kind="ExternalInput")
    w1_d = nc.dram_tensor("w1", [128, 2, F], dt.bfloat16, kind="ExternalInput")
    w2_d = nc.dram_tensor("w2", [128, F], dt.bfloat16, kind="ExternalInput")
    b1_d = nc.dram_tensor("b1", [128, 1], dt.float32, kind="ExternalInput")
    b2_d = nc.dram_tensor("b2", [128, 1], dt.float32, kind="ExternalInput")
    outT = nc.dram_tensor("outT", [F, B], dt.float32, kind="ExternalOutput")

    with tile.TileContext(nc) as tc:
        with (
            tc.tile_pool(name="const", bufs=1) as constp,
            tc.tile_pool(name="xt", bufs=PREFETCH) as xtp,
            tc.tile_pool(name="gates", bufs=3) as gatep,
        ):
            # ---------------- weights / constants ----------------
            # first chunk's first step slice goes FIRST (sync queue) and the
            # weights load in parallel on the scalar HWDGE queue, so step 0
            # starts while the rest of chunk 0 is still in flight
            xt0 = xtp.tile([128, 3, TC, B], dt.bfloat16, tag="xt", name="xt_0")
            nc.sync.dma_start(xt0[:, :, 0:1, :], xt_d[0][:, :, 0:1, :])
            wi_bf = constp.tile([128, 3, G], dt.bfloat16)
            nc.scalar.dma_start(wi_bf[:], wi_d[:])
            nc.sync.dma_start(xt0[:, :, 1:TC, :], xt_d[0][:, :, 1:TC, :])
            wh_bf = constp.tile([128, G], dt.bfloat16)
            nc.scalar.dma_start(wh_bf[:], wh_d[:])
            w1_bf = constp.tile([128, 2, F], dt.bfloat16)
            nc.scalar.dma_start(w1_bf[:], w1_d[:])
            w2_bf = constp.tile([128, F], dt.bfloat16)
            nc.scalar.dma_start(w2_bf[:], w2_d[:])
            b1t = constp.tile([128, 1], dt.float32)
            nc.scalar.dma_start(b1t[:], b1_d[:])
            b2t = constp.tile([128, 1], dt.float32)
            nc.scalar.dma_start(b2t[:], b2_d[:])
            srcT = constp.tile([128, B], dt.bfloat16)
            nc.scalar.dma_start(srcT[:], srcT_d[:])
            bias_g = constp.tile([128, 4], dt.float32)
            nc.scalar.dma_start(bias_g[:], bhq_d[:])

            # persistent state: cell (bf16) + hidden (bf16), per half
            cs = constp.tile([128, 2, NH], dt.bfloat16)
            hs = constp.tile([128, 2, NH], dt.bfloat16)

            # xt chunk tiles (named, managed as a rotating set)
            def xt_tile(ch):
                return xtp.tile(
                    [128, 3, TC, B], dt.bfloat16, tag="xt", name=f"xt_{ch}"
                )

            xt_tiles = {}

            def load_chunk(ch):
                xt_tiles[ch] = xt_tile(ch)
                nc.sync.dma_start(xt_tiles[ch][:], xt_d[ch])

            # ---------------- per-step pieces ----------------
            zp_ctx = tc.tile_pool(name="zp", bufs=2, space="PSUM")
            zp = zp_ctx.__enter__()
            z_tiles = [None, None]

            def inproj(h, t):
                """12 MMs: z[h] = Wi^T x_t  (start of PSUM accumulation)."""
                z = zp.tile([128, 4, NH], dt.float32, tag=f"z{h}", name=f"z{h}_{t}")
                z_tiles[h] = z
                ch, tt = t // TC, t % TC
                xt_ = xt_tiles[ch]
                last = t == 0  # no rec matmuls at t==0
                for q in range(4):
                    for kc in range(3):
                        nc.tensor.matmul(
                            z[:, q, :],
                            wi_bf[:, kc, q * 128 : (q + 1) * 128],
                            xt_[:, kc, tt, h * NH : (h + 1) * NH],
                            start=(kc == 0 and q in (QG, QI)),
                            stop=(last and kc == 2 and q in (QF, QO)),
                        )

            def rec_mms(h, t):
                """4 MMs: z[h] += Wh^T h  (order g,f,i,o; closes accumulation)."""
                z = z_tiles[h]
                for q in range(4):
                    nc.tensor.matmul(
                        z[:, q, :],
                        wh_bf[:, q * 128 : (q + 1) * 128],
                        hs[:, h, :],
                        start=False,
                        stop=(q in (QF, QO)),
                    )

            def gates(h, t):
                """ACT: one sigmoid over all 4 quads (g-quad pre-scaled 2x,
                so sg[:,0] = sigmoid(2g) and tanh(g) = 2*sg[:,0]-1)."""
                z = z_tiles[h]
                sg = gatep.tile(
                    [128, 4, NH], dt.bfloat16, tag=f"sg{h}", name=f"sg{h}_{t}"
                )
                if zero_bias:
                    nc.scalar.activation(sg[:], z[:], AF.Sigmoid)
                else:
                    for q in range(4):
                        nc.scalar.activation(
                            sg[:, q, :],
                            z[:, q, :],
                            AF.Sigmoid,
                            bias=bias_g[:, q : q + 1],
                        )
                return sg

            def cell_tail(h, t, sg):
                """DVE cell update + ACT tanh(c) + DVE h-mul.
                sg quads: [0]=sigmoid(2g), [1]=f, [2]=i, [3]=o."""
                tg = gatep.tile([128, NH], dt.bfloat16, tag=f"tg{h}", name=f"tg{h}_{t}")
                nc.vector.tensor_scalar(
                    tg[:], sg[:, 0, :], 2.0, -1.0,
                    mybir.AluOpType.mult, mybir.AluOpType.add,
                )
                if t == 0:
                    # c = sig(i)*tanh(g) directly (f-term is zero)
                    nc.vector.tensor_mul(cs[:, h, :], sg[:, 2, :], tg[:])
                else:
                    m2 = gatep.tile(
                        [128, NH], dt.bfloat16, tag=f"m2_{h}", name=f"m2_{h}_{t}"
                    )
                    nc.vector.tensor_mul(m2[:], sg[:, 2, :], tg[:])
                    m1 = gatep.tile(
                        [128, NH], dt.bfloat16, tag=f"m1_{h}", name=f"m1_{h}_{t}"
                    )
                    nc.vector.tensor_mul(m1[:], sg[:, 1, :], cs[:, h, :])
                    nc.vector.tensor_add(cs[:, h, :], m1[:], m2[:])
                tc2 = gatep.tile(
                    [128, NH], dt.bfloat16, tag=f"tc2_{h}", name=f"tc2_{h}_{t}"
                )
                nc.scalar.activation(tc2[:], cs[:, h, :], AF.Tanh)
                nc.vector.tensor_mul(hs[:, h, :], sg[:, 3, :], tc2[:])

            # ---------------- main loop (software-pipelined) ----------------
            xt_tiles[0] = xt0
            for ch in range(1, PREFETCH):
                load_chunk(ch)
            inproj(0, 0)

            s0 = s1 = None
            for t in range(T):
                if t % TC == 0 and t // TC + PREFETCH < NCHUNK:
                    load_chunk(t // TC + PREFETCH)
                # P1: front h0
                if t > 0:
                    rec_mms(0, t)
                # P2: gates h0
                s0 = gates(0, t)
                # P3: cell+tail h1 (step t-1)
                if t > 0:
                    cell_tail(1, t - 1, s1)
                # P4: inproj h1 (step t)
                inproj(1, t)
                # P5: front h1
                if t > 0:
                    rec_mms(1, t)
                # P6: gates h1
                s1 = gates(1, t)
                # P7: cell+tail h0 (step t)
                cell_tail(0, t, s0)
                # P8: inproj h0 (step t+1)
                if t + 1 < T:
                    inproj(0, t + 1)
            cell_tail(1, T - 1, s1)

            zp_ctx.__exit__(None, None, None)

            # ---------------- merge layer ----------------
            with tc.tile_pool(name="mp", bufs=1, space="PSUM") as mp:
                ps_hid = mp.tile([128, B], dt.float32)
                nc.tensor.matmul(
                    ps_hid[:], w1_bf[:, 0, :], hs[:].rearrange("p h n -> p (h n)"),
                    start=True, stop=False,
                )
                nc.tensor.matmul(
                    ps_hid[:], w1_bf[:, 1, :], srcT[:], start=False, stop=True
                )
                hid_bf = constp.tile([128, B], dt.bfloat16)
                nc.scalar.activation(hid_bf[:], ps_hid[:], AF.Relu, bias=b1t[:])

                ps_out = mp.tile([128, B], dt.float32)
                nc.tensor.matmul(ps_out[:], w2_bf[:], hid_bf[:], start=True, stop=True)
                out_sb = constp.tile([128, B], dt.float32)
                nc.scalar.activation(out_sb[:], ps_out[:], AF.Identity, bias=b2t[:])
                nc.sync.dma_start(outT[:], out_sb[:])

    nc.compile()
    return nc


_NC_CACHE: dict = {}


def _get_nc(zero_bias: bool):
    if zero_bias not in _NC_CACHE:
        _NC_CACHE[zero_bias] = build_nc(zero_bias)
    return _NC_CACHE[zero_bias]


def make_in_maps(**inputs):
    """Host-side prep: shard batch, transpose+cast x to bf16 device layout."""
    bf16 = ml_dtypes.bfloat16
    f32 = lambda x: np.asarray(x, dtype=np.float32)

    Wi = f32(inputs["Wi"])  # [384, 512]
    Wh = f32(inputs["Wh"])  # [128, 512]
    bh = f32(inputs["bh"])  # [512]
    W1 = f32(inputs["W1"])  # [256, 128]
    W2 = f32(inputs["W2"])  # [128, 128]
    b1 = f32(inputs["b1"])
    b2 = f32(inputs["b2"])

    # permute gate quads to device order [g, f, i, o]; scale the g-quad by
    # 2 (exact in bf16) so tanh(g) = 2*sigmoid(2g)-1 needs only a sigmoid
    qcols = np.concatenate([np.arange(q * F, (q + 1) * F) for q in QPERM])
    Wi_p = Wi[:, qcols].copy()
    Wh_p = Wh[:, qcols].copy()
    bh_p = bh[qcols].copy()
    Wi_p[:, :F] *= 2.0
    Wh_p[:, :F] *= 2.0
    bh_p[:F] *= 2.0

    wi_host = np.ascontiguousarray(
        Wi_p.reshape(3, 128, G).transpose(1, 0, 2).astype(bf16)
    )  # [128, 3, G]
    wh_host = Wh_p.astype(bf16)  # [128, G]
    bhq_host = np.ascontiguousarray(bh_p.reshape(4, 128).T)  # [128, 4] fp32
    w1_host = np.ascontiguousarray(
        W1.reshape(2, 128, F).transpose(1, 0, 2).astype(bf16)
    )  # [128, 2, F]
    w2_host = W2.astype(bf16)  # [128, F]
    b1_host = b1.reshape(F, 1).astype(np.float32)
    b2_host = b2.reshape(F, 1).astype(np.float32)

    # x: [3, B, T, F] -> bf16 -> per-core [NCHUNK, 128f, 3, TC, Bc]
    X = np.stack(
        [f32(inputs["seq"]), f32(inputs["seq_e"]), f32(inputs["seq_t"])], axis=0
    ).astype(bf16)
    src = f32(inputs["src"]).astype(bf16)

    shared = dict(
        wi=wi_host, wh=wh_host, bhq=bhq_host, w1=w1_host, w2=w2_host,
        b1=b1_host, b2=b2_host,
    )
    in_maps = []
    for c in range(NCORES):
        sl = slice(c * B, (c + 1) * B)
        xc = X[:, sl]  # [3, B, T, F]
        # -> [NCHUNK, F, 3, TC, B]
        xc = xc.reshape(3, B, NCHUNK, TC, F).transpose(2, 4, 0, 3, 1)
        m = dict(shared)
        m["xt"] = np.ascontiguousarray(xc)
        m["srcT"] = np.ascontiguousarray(src[sl].T)  # [F, B]
        in_maps.append(m)
    return in_maps


def kernel(**inputs) -> np.ndarray:
    zero_bias = not np.any(np.asarray(inputs["bh"]))
    nc = _get_nc(zero_bias)
    in_maps = make_in_maps(**inputs)
    res = run_bass_kernel_spmd(nc, in_maps, core_ids=list(range(NCORES)))
    out = np.empty((BFULL, F), np.float32)
    for c in range(NCORES):
        out[c * B : (c + 1) * B] = res.results[c]["outT"].T
    return out
